# revision 8
# baseline (speedup 1.0000x reference)
"""Grouped multivariate kernel-CRPS loss on 8 TRN2 NeuronCores.

Sharding: latlon (20480) split across 8 cores (2560 each). Per core, one
mega-tile per (b,t): 128 partitions x 20 latlon points each. All 36 unique
pair diffs per point via the circular-distance trick (d=1..3 full, d=4 half),
|w|^1.5 = exp(0.75 ln(w^2)), grouped sums via one native reduce, final
S^(2/3) with the 1/8 and -1/56 weights folded into the Exp bias. This
environment has a large fixed per-instruction cost, so the kernel uses few,
very wide instructions (~72 total).
"""
import sys
sys.path.insert(0, '/opt/trn_rl_repo')
import math
import numpy as np
import ml_dtypes

import concourse.bacc as bacc
import concourse.mybir as mybir
from concourse.tile import TileContext
from concourse.bass_utils import run_bass_kernel_spmd
import bass_rust

F32 = mybir.dt.float32
BF16 = mybir.dt.bfloat16
Alu = mybir.AluOpType
Act = mybir.ActivationFunctionType

B, E, T, LATLON, K = 2, 8, 2, 20480, 32
NCORES = 8
SHARD = LATLON // NCORES          # 2560
LPP = SHARD // 128                # 20 latlon points per partition
SLK = LPP * K                     # 640: one ensemble slot per partition
GRP = 36 * LPP                    # 720 groups per tile
WW = GRP * K                      # 23040 wide elems per partition per tile
NT = B * T                        # 4 tiles per core

_CACHE = {}


def _ap(base, pairs, off):
    c = base.copy()
    c.ap = bass_rust.VecI64Pair(pairs)
    c.offset = off
    return c


def build(reps=1):
    key = ('nc', reps)
    if key in _CACHE:
        return _CACHE[key]
    nc = bacc.Bacc()
    preds = nc.dram_tensor("preds", [B, E, T, SHARD, K], BF16, kind="ExternalInput")
    target = nc.dram_tensor("target", [B, 1, T, SHARD, K], BF16, kind="ExternalInput")
    fw = nc.dram_tensor("fw", [K], F32, kind="ExternalInput")
    nwc = nc.dram_tensor("nwc", [SHARD], F32, kind="ExternalInput")
    out = nc.dram_tensor("out", [1, 1], F32, kind="ExternalOutput")

    with TileContext(nc) as tc:
        with tc.tile_pool(name="const", bufs=1) as cp, \
             tc.tile_pool(name="work", bufs=2) as wp, \
             tc.tile_pool(name="big", bufs=1) as bp, \
             tc.tile_pool(name="acc", bufs=1) as ap_, \
             tc.tile_pool(name="ps", bufs=1, space="PSUM") as ps:
            FW = cp.tile([128, K], F32, tag="FW")
            nc.gpsimd.dma_start(out=FW[:], in_=fw[:].partition_broadcast(128))
            NWT = cp.tile([128, LPP], F32, tag="NWT")
            nc.sync.dma_start(out=NWT[:], in_=nwc[:].rearrange("(p l) -> p l", p=128))
            BIASE = cp.tile([128, 1], F32, tag="BIASE")
            nc.vector.memset(BIASE[:], math.log(1.0 / 8.0))
            BIASD = cp.tile([128, 1], F32, tag="BIASD")
            nc.vector.memset(BIASD[:], math.log(1.0 / 56.0))
            EPSB = cp.tile([128, 1], F32, tag="EPSB")
            nc.vector.memset(EPSB[:], 1e-30)
            SACC = ap_.tile([128, NT * GRP], F32, tag="SACC")

            for rep in range(reps):
                for bt in range(B * T):
                    b, t = bt // T, bt % T
                    Praw = wp.tile([128, E * SLK], BF16, tag="Praw")
                    nc.sync.dma_start(out=Praw[:], in_=_ap(
                        preds[:], [(SLK, 128), (T * SHARD * K, E), (1, SLK)],
                        (b * E * T + t) * SHARD * K))
                    Yraw = wp.tile([128, SLK], BF16, tag="Yraw")
                    nc.sync.dma_start(out=Yraw[:], in_=_ap(
                        target[:], [(SLK, 128), (1, SLK)],
                        (b * T + t) * SHARD * K))

                    P2 = wp.tile([128, 12 * SLK], BF16, tag="P2")
                    nc.vector.tensor_tensor(
                        P2[:, 0:E * SLK].rearrange("p (e l k) -> p e l k", e=E, k=K),
                        Praw[:].rearrange("p (e l k) -> p e l k", e=E, k=K),
                        _ap(FW[:], [(K, 128), (0, E), (0, LPP), (1, K)], 0),
                        Alu.mult)
                    nc.scalar.copy(P2[:, E * SLK:12 * SLK], P2[:, 0:4 * SLK])
                    YB = wp.tile([128, SLK], BF16, tag="YB")
                    nc.vector.tensor_tensor(
                        YB[:].rearrange("p (l k) -> p l k", k=K),
                        Yraw[:].rearrange("p (l k) -> p l k", k=K),
                        _ap(FW[:], [(K, 128), (0, LPP), (1, K)], 0),
                        Alu.mult)

                    W = bp.tile([128, WW], BF16, tag="W")
                    nc.vector.tensor_tensor(
                        W[:, 0:E * SLK].rearrange("p (e l k) -> p e l k", e=E, k=K),
                        _ap(YB[:], [(SLK, 128), (0, E), (K, LPP), (1, K)], 0),
                        P2[:, 0:E * SLK].rearrange("p (e l k) -> p e l k", e=E, k=K),
                        Alu.subtract)
                    for d in (1, 2, 3):
                        nc.vector.tensor_tensor(
                            W[:, d * E * SLK:(d + 1) * E * SLK]
                            .rearrange("p (i l k) -> p i l k", i=E, k=K),
                            _ap(P2[:], [(12 * SLK, 128), (SLK, E), (K, LPP), (1, K)], 0),
                            _ap(P2[:], [(12 * SLK, 128), (SLK, E), (K, LPP), (1, K)], d * SLK),
                            Alu.subtract)
                    o4 = 4 * E * SLK
                    nc.vector.tensor_tensor(
                        W[:, o4:o4 + 4 * SLK].rearrange("p (i l k) -> p i l k", i=4, k=K),
                        _ap(P2[:], [(12 * SLK, 128), (SLK, 4), (K, LPP), (1, K)], 0),
                        _ap(P2[:], [(12 * SLK, 128), (SLK, 4), (K, LPP), (1, K)], 4 * SLK),
                        Alu.subtract)

                    WA = bp.tile([128, WW], BF16, tag="WA")
                    nc.vector.tensor_tensor(WA[:], W[:], W[:], Alu.mult)
                    nc.scalar.activation(W[:], WA[:], Act.Ln, bias=EPSB[:])
                    nc.scalar.activation(WA[:], W[:], Act.Exp, scale=0.75)

                    nc.vector.tensor_reduce(
                        SACC[:, bt * GRP:(bt + 1) * GRP],
                        WA[:].rearrange("p (g k) -> p g k", k=K),
                        axis=mybir.AxisListType.X, op=Alu.add)

            LNS = ap_.tile([128, NT * GRP], F32, tag="LNS")
            nc.scalar.activation(LNS[:], SACC[:], Act.Ln)
            NPW = ap_.tile([128, NT * GRP], F32, tag="NPW")
            t3 = NPW[:].rearrange("p (t g) -> p t g", g=GRP)
            l3 = LNS[:].rearrange("p (t g) -> p t g", g=GRP)
            EC = E * LPP
            nc.scalar.activation(t3[:, :, 0:EC], l3[:, :, 0:EC],
                                 Act.Exp, scale=2.0 / 3.0, bias=BIASE[:])
            nc.scalar.activation(t3[:, :, EC:GRP], l3[:, :, EC:GRP],
                                 Act.Exp, scale=2.0 / 3.0, bias=BIASD[:])
            nc.vector.tensor_scalar(
                t3[:, :, EC:GRP], t3[:, :, EC:GRP], -1.0, None, Alu.mult)
            KW = ap_.tile([128, NT * GRP], F32, tag="KW")
            nc.vector.tensor_tensor(
                KW[:].rearrange("p (t g l) -> p t g l", t=NT, l=LPP),
                NPW[:].rearrange("p (t g l) -> p t g l", t=NT, l=LPP),
                _ap(NWT[:], [(LPP, 128), (0, NT), (0, 36), (1, LPP)], 0),
                Alu.mult)
            GR = ap_.tile([128, 1], F32, tag="GR")
            nc.vector.tensor_reduce(GR[:], KW[:], axis=mybir.AxisListType.X, op=Alu.add)
            ones = ap_.tile([128, 1], F32, tag="ones")
            nc.vector.memset(ones[:], 1.0)
            pt = ps.tile([1, 1], F32, tag="pt")
            nc.tensor.matmul(pt[:], ones[:], GR[:], start=True, stop=True)
            FIN = ap_.tile([1, 1], F32, tag="FIN")
            nc.vector.tensor_copy(FIN[:], pt[:])
            nc.sync.dma_start(out=out[:, :], in_=FIN[:])
    nc.finalize()
    _CACHE[key] = nc
    return nc


def kernel(preds, target, node_weights, feature_weights, _trace=False, _reps=1, **kw):
    nc = build(_reps)
    pb = preds.astype(ml_dtypes.bfloat16)
    tb = target.astype(ml_dtypes.bfloat16)
    nwf = node_weights.astype(np.float32)
    fwf = (feature_weights / feature_weights.size).astype(np.float32)
    in_maps = []
    for c in range(NCORES):
        s = slice(c * SHARD, (c + 1) * SHARD)
        in_maps.append({
            "preds": np.ascontiguousarray(pb[:, :, :, s, :]),
            "target": np.ascontiguousarray(tb[:, :, :, s, :]),
            "fw": fwf,
            "nwc": np.ascontiguousarray(nwf[s]),
        })
    res = run_bass_kernel_spmd(nc, in_maps, core_ids=list(range(NCORES)))
    total = sum(float(r["out"][0, 0]) for r in res.results)
    total = total / float(nwf.sum()) / B
    return np.float32(total)
